# revision 58
# baseline (speedup 1.0000x reference)
"""Trainium2 Bass kernel for nn_CausalLayer (bilinear causal mixing layer).

Math (per batch b):
    E = ae[x]                                # [L, D] gather
    S[i,j] = E_i @ w @ E_j                   # bilinear pairwise score
    coef[i,j] = (i+1)/(j+1) for i<j else 0
    res[:,j] = bx[:,j] + sum_i coef[i,j]*S[i,j]*bx[:,i]

Chunked linear-attention identity (chunk C=128). With a_i = w^T E_i,
e'_j = E_j/(j+1) and y_i = (i+1)*bx_i:

    acc_j = M_cj @ e'_j + sum_{i<j, same chunk} ((i+1) a_i . e'_j) bx_i
    M_c   = sum_{i in chunks < c} y_i a_i^T      (rank-D running state, [D, H])
    res_j = bx_j + acc_j                          (final add on host)

Host prep (all O(L*D)): the ae gather, A = E @ w, the (i+1)/(1/(j+1)) row
and column scalings, and the [D, C] transposes. The device streams three
[*,768]-wide matmul groups plus one [128,128] score block per chunk — the
structural PE floor for this decomposition (~19 PE columns/token).

Schedule: per chunk the PE runs [M-update, S(next), out1, out2], the score
block software-pipelined one chunk ahead. The rank-D state M lives folded as
[128, 512] PSUM (H split 512/256 on partition halves, Et duplicated to
partitions 64-127 for the second half) so its bf16 snapshot is one cheap
Act-engine copy — the Act queue carries nothing else, so the snapshot is
always ready a full chunk before out2 needs it. The acc epilogue is a single
PSUM->bf16 copy on the DVE. Loads move 4 chunks per descriptor batch on the
Sync queue (prefetched two groups ahead, chunk 0 fast-pathed across queues);
stores ride the otherwise-idle GpSimd queue, per-chunk for the final group
so the post-loop drain is one small transfer. bf16 in and out with f32 PSUM
accumulation, ~4.5e-3 max scale-relative error vs the fp32 reference.

Sharding: batch-parallel, 2 of 16 batches per NeuronCore across 8 cores;
all tables are per-core slices. No cross-core communication.
"""

import os
import sys

for _p in ("/opt/trn_rl_repo", "/root/.axon_site/_ro/trn_rl_repo"):
    if os.path.isdir(_p) and _p not in sys.path:
        sys.path.insert(0, _p)

import numpy as np

B, L, H = 16, 2048, 768
V, D = 30000, 64
NCORES = 8
BPC = B // NCORES          # batches per core
C = 128                    # chunk (tile) size along sequence
NCH = L // C               # chunks per batch
ROWS = BPC * L             # bx rows per core
NT = BPC * NCH             # total chunks per core
G = 4                      # chunks per DMA group
HLO = 512                  # H split: [0:512] on partitions 0:64, [512:768] above
HHI = H - HLO

_compiled = {}


def _build():
    """Build + compile the per-core Bass module (SPMD: same program, 8 cores)."""
    key = "v4"
    if key in _compiled:
        return _compiled[key]

    import concourse.bacc as bacc
    import concourse.bass as bass
    import concourse.mybir as mybir
    import concourse.tile as tile

    f32 = mybir.dt.float32
    bf16 = mybir.dt.bfloat16

    nc = bacc.Bacc(
        "TRN2",
        target_bir_lowering=False,
        debug=False,
        enable_asserts=False,
        num_devices=NCORES,
    )

    bx_d = nc.dram_tensor("bx", [ROWS, H], bf16, kind="ExternalInput").ap()
    # per global chunk g: cols [g*2C, g*2C+C) = Et'_g ([D, C] transposed E,
    # column j scaled by 1/(j+1)), cols [g*2C+C, (g+1)*2C) = At'_g
    # ((i+1)-scaled transposed A)
    etat_d = nc.dram_tensor("etat", [D, 2 * ROWS], bf16, kind="ExternalInput").ap()
    # Ap rows aligned with bx rows: row i = (i+1) * a_i
    ap_d = nc.dram_tensor("ap", [ROWS, D], bf16, kind="ExternalInput").ap()
    um_d = nc.dram_tensor("umask", [C, C], f32, kind="ExternalInput").ap()
    out_d = nc.dram_tensor("out", [ROWS, H], bf16, kind="ExternalOutput").ap()

    mult = mybir.AluOpType.mult

    with tile.TileContext(nc) as tc:
        with (
            tc.tile_pool(name="const", bufs=1) as cpool,
            tc.tile_pool(name="bxp", bufs=4) as bxpool,
            tc.tile_pool(name="outp", bufs=3) as outpool,
            tc.tile_pool(name="eat", bufs=4) as eatpool,
            tc.tile_pool(name="app", bufs=4) as appool,
            tc.tile_pool(name="sm", bufs=4) as smpool,
            tc.tile_pool(name="mp", bufs=2) as mpool,
            tc.tile_pool(name="ps_s", bufs=2, space="PSUM") as ps_s,
            tc.tile_pool(name="ps_out", bufs=2, space="PSUM") as ps_out,
            tc.tile_pool(name="ps_m", bufs=2, space="PSUM") as ps_m,
        ):
            umask_s = cpool.tile([C, C], f32)

            bx_t = [None] * NT
            eat_t = [None] * NT
            ap_t = [None] * NT

            def load_group(gr, eng=None):
                """DMA one group of G chunks (bx / etat / ap).

                etat lands duplicated on both partition halves: the lower
                copy feeds the score block and out2-lo, the upper copy is
                the out2-hi stationary (PE array rows 64:128). Group 0 skips
                chunk 0, which the prologue fast-path loads separately so
                the pipeline starts without waiting on a full group."""
                eng = eng or nc.sync
                t0 = gr * G
                sk = 1 if gr == 0 else 0
                n = G - sk
                EAT4 = eatpool.tile(
                    [2 * D, G * 2 * C], bf16, name=f"EAT4_{gr}", tag="EAT4"
                )
                src = etat_d[:, (t0 + sk) * 2 * C:(t0 + G) * 2 * C]
                eng.dma_start(out=EAT4[0:D, sk * 2 * C:], in_=src)
                eng.dma_start(out=EAT4[D:2 * D, sk * 2 * C:], in_=src)
                AP4 = appool.tile([C, G * D], bf16, name=f"AP4_{gr}", tag="AP4")
                eng.dma_start(
                    out=AP4[:, sk * D:].rearrange("p (g d) -> p g d", g=n),
                    in_=ap_d[(t0 + sk) * C:(t0 + G) * C, :].rearrange(
                        "(g p) d -> p g d", g=n
                    ),
                )
                BX4 = bxpool.tile([C, G * H], bf16, name=f"BX4_{gr}", tag="BX4")
                eng.dma_start(
                    out=BX4[:, sk * H:].rearrange("p (g h) -> p g h", g=n),
                    in_=bx_d[(t0 + sk) * C:(t0 + G) * C, :].rearrange(
                        "(g p) h -> p g h", g=n
                    ),
                )
                for q in range(sk, G):
                    t = t0 + q
                    bx_t[t] = BX4[:, q * H:(q + 1) * H]
                    eat_t[t] = EAT4
                    ap_t[t] = AP4[:, q * D:(q + 1) * D]

            def s_block(t):
                """Score block S'(t) on PE + mask on DVE (pipelined ahead)."""
                q = t % G
                Atp = eat_t[t][0:D, q * 2 * C + C:(q + 1) * 2 * C]
                Etp = eat_t[t][0:D, q * 2 * C:q * 2 * C + C]
                s_p = ps_s.tile([C, C], f32, name=f"s_p_{t}", tag="s_p")
                nc.tensor.matmul(
                    out=s_p[:], lhsT=Atp, rhs=Etp, start=True, stop=True,
                )
                St = smpool.tile([C, C], bf16, name=f"St_{t}", tag="St")
                nc.vector.tensor_tensor(
                    out=St[:], in0=s_p[:], in1=umask_s[:], op=mult,
                )
                return St

            # fast-path chunk 0: tiny dedicated loads so the score block and
            # the first out1/M-update start as early as possible; the posts
            # are spread across queues so they configure DGE concurrently
            # instead of serializing on the Sync sequencer
            eat0 = cpool.tile([2 * D, 2 * C], bf16)
            nc.scalar.dma_start(out=eat0[0:D, :], in_=etat_d[:, 0:2 * C])
            nc.scalar.dma_start(out=eat0[D:2 * D, :], in_=etat_d[:, 0:2 * C])
            bx0 = cpool.tile([C, H], bf16)
            nc.sync.dma_start(out=bx0[:], in_=bx_d[0:C, :])
            ap0 = cpool.tile([C, D], bf16)
            nc.gpsimd.dma_start(out=ap0[:], in_=ap_d[0:C, :])
            nc.sync.dma_start(out=umask_s[:], in_=um_d[:, :])
            bx_t[0] = bx0[:, :]
            eat_t[0] = eat0
            ap_t[0] = ap0[:, :]

            St_next = s_block(0)
            load_group(0)
            load_group(1)

            out_ps = [None] * NT
            out4s = [None] * (NT // G)

            def acc_hi_and_store(u):
                """Act-engine copy of out_p(u)'s high half, phase-shifted one
                chunk behind the DVE low half: by the time the Act queue
                reaches it the PSUM is long complete, so it never blocks the
                M snapshots ahead of it in the queue. The store for a chunk
                group is posted right after the hi-copy that completes it."""
                qq = u % G
                O4 = out4s[u // G]
                nc.scalar.copy(
                    out=O4[:, qq * H + HLO:(qq + 1) * H],
                    in_=out_ps[u][:, HLO:H],
                )
                if u >= NT - G:
                    # final group: store per chunk so the drain after the
                    # last copy is one small transfer, not 4 chunks
                    nc.gpsimd.dma_start(
                        out=out_d[u * C:(u + 1) * C, :],
                        in_=O4[:, qq * H:(qq + 1) * H],
                    )
                elif qq == G - 1:
                    u0 = u - G + 1
                    nc.gpsimd.dma_start(
                        out=out_d[u0 * C:(u + 1) * C, :].rearrange(
                            "(g p) h -> p g h", g=G
                        ),
                        in_=O4[:].rearrange("p (g h) -> p g h", g=G),
                    )

            M_p = None
            M_s = None
            for t in range(NT):
                b, c = divmod(t, NCH)
                q = t % G
                BX = bx_t[t]
                Etp = eat_t[t][0:D, q * 2 * C:q * 2 * C + C]
                EtpD = eat_t[t][D:2 * D, q * 2 * C:q * 2 * C + C]

                if q == 0 and t // G + 2 < NT // G:
                    load_group(t // G + 2)

                if c == 0:
                    # folded rank-D state: partitions 0:64 hold M[:, 0:512],
                    # partitions 64:128 hold M[:, 512:768] (cols 256:512 of
                    # the upper half are dead; zero them once so the bf16
                    # snapshot below never reads uninitialized PSUM)
                    M_p = ps_m.tile([2 * D, HLO], f32, name=f"M_p_b{b}", tag="M_p")
                    nc.vector.memset(M_p[D:2 * D, HHI:HLO], 0.0)

                # M += y^T-outer-a, folded  (skip the never-read last update).
                # skip_group_check: the sim's group guard can't express this
                # read-between-accumulations pattern; the pending-zero
                # accumulate semantics and Tile's HW sync are unaffected.
                if c < NCH - 1:
                    nc.tensor.matmul(
                        out=M_p[0:D, 0:HLO],
                        lhsT=ap_t[t],
                        rhs=BX[:, 0:HLO],
                        start=(c == 0),
                        stop=True,
                        skip_group_check=True,
                    )
                    nc.tensor.matmul(
                        out=M_p[D:2 * D, 0:HHI],
                        lhsT=ap_t[t],
                        rhs=BX[:, HLO:H],
                        start=(c == 0),
                        stop=True,
                        skip_group_check=True,
                    )

                St = St_next
                if t + 1 < NT:
                    St_next = s_block(t + 1)

                # acc = St^T @ BX (+ Et'^T @ M)  [C, H]
                out_p = ps_out.tile([C, H], f32, name=f"out_p_{t}", tag="out_p")
                for lo, hi in ((0, HLO), (HLO, H)):
                    nc.tensor.matmul(
                        out=out_p[:, lo:hi],
                        lhsT=St[:],
                        rhs=BX[:, lo:hi],
                        start=True,
                        stop=(c == 0),
                    )
                if c > 0:
                    nc.tensor.matmul(
                        out=out_p[:, 0:HLO],
                        lhsT=Etp,
                        rhs=M_s[0:D, 0:HLO],
                        start=False,
                        stop=True,
                    )
                    nc.tensor.matmul(
                        out=out_p[:, HLO:H],
                        lhsT=EtpD,
                        rhs=M_s[D:2 * D, 0:HHI],
                        start=False,
                        stop=True,
                    )

                # snapshot M for the NEXT chunk (reads M_p after this chunk's
                # update, before the next one; the Act engine runs it as soon
                # as the update's semaphore fires, independent of issue order)
                if t + 1 < NT and (t + 1) % NCH != 0:
                    M_s = mpool.tile([2 * D, HLO], bf16, name=f"M_s_{t + 1}", tag="M_s")
                    nc.scalar.copy(out=M_s[:], in_=M_p[:])

                # acc -> bf16 out tile: DVE copies the low half now, the Act
                # engine picks up the high half one chunk later (see
                # acc_hi_and_store); grouped stores ride the otherwise-idle
                # GpSimd queue so the Sync queue's load posts never block
                # behind compute
                if q == 0:
                    OUT4 = outpool.tile([C, G * H], bf16, name=f"OUT4_{t}", tag="OUT4")
                    out4s[t // G] = OUT4
                nc.vector.tensor_scalar_add(
                    out=OUT4[:, q * H:q * H + HLO],
                    in0=out_p[:, 0:HLO],
                    scalar1=0.0,
                )
                out_ps[t] = out_p
                if t > 0:
                    acc_hi_and_store(t - 1)
            acc_hi_and_store(NT - 1)

    # Adjacent PE matmuls often share a stationary operand (the two H-halves
    # of out1); legalization has already paired each matmul with a standalone
    # InstLdweights, so drop the redundant reloads. The key includes the PE
    # array tile position: the same weights loaded into a different array
    # quadrant is a genuine reload.
    ndropped = 0
    for blk in nc.m.functions[0].blocks:
        keep = []
        last_w = None
        for inst in blk.instructions:
            if getattr(inst, "engine", None) == mybir.EngineType.PE:
                if isinstance(inst, mybir.InstLdweights):
                    w = inst.ins[0]
                    wkey = (
                        w.memref,
                        w.offset,
                        str(w.ap),
                        str(getattr(inst, "tile_position", None)),
                        str(getattr(inst, "tile_size", None)),
                    )
                    if (
                        last_w is not None
                        and wkey == last_w
                        and not inst.has_wait()
                    ):
                        ndropped += 1
                        continue
                    last_w = wkey
                elif not isinstance(inst, mybir.InstMatmult):
                    last_w = None
            keep.append(inst)
        blk.instructions = keep
    if os.environ.get("BASS_DEBUG_FUSE"):
        print(f"[kernel] redundant ldweights dropped: {ndropped}", file=sys.stderr)

    nc.compile()
    _compiled[key] = nc
    return nc


def _np_umask():
    i = np.arange(C)
    return (i[:, None] < i[None, :]).astype(np.float32)


def _in_maps(bert_x, x, ae, w):
    import ml_dtypes

    bf16 = ml_dtypes.bfloat16
    bert_x = np.asarray(bert_x, dtype=np.float32)
    x = np.asarray(x)
    ae = np.asarray(ae, dtype=np.float32)
    w = np.asarray(w, dtype=np.float32)

    E = ae[x.reshape(-1)]                     # [B*L, D]
    A = E @ w                                 # [B*L, D]
    jp1 = (np.arange(L, dtype=np.float64) + 1.0).astype(np.float32)
    Ap = (A.reshape(B, L, D) * jp1[None, :, None]).reshape(B * L, D)
    Einv = (E.reshape(B, L, D) / jp1[None, :, None]).reshape(B * L, D)

    bx16 = np.ascontiguousarray(bert_x.reshape(B * L, H).astype(bf16))
    ap16 = np.ascontiguousarray(Ap.astype(bf16))

    # etat per core: [D, 2*ROWS]; per global chunk g: [Et'_g | At'_g]
    Ech = Einv.reshape(B, NCH, C, D).astype(bf16)
    Ach = Ap.reshape(B, NCH, C, D).astype(bf16)
    pair = np.stack([Ech, Ach], axis=2)       # [B, NCH, 2, C, D]
    pair = pair.transpose(0, 4, 1, 2, 3)      # [B, D, NCH, 2, C]

    umask = _np_umask()
    maps = []
    for k in range(NCORES):
        et = np.ascontiguousarray(
            pair[k * BPC:(k + 1) * BPC].transpose(1, 0, 2, 3, 4).reshape(D, 2 * ROWS)
        )
        maps.append(
            {
                "bx": bx16[k * BPC * L:(k + 1) * BPC * L],
                "etat": et,
                "ap": ap16[k * BPC * L:(k + 1) * BPC * L],
                "umask": umask,
            }
        )
    return maps


def _run(bert_x, x, ae, w, trace=False):
    from concourse import bass_utils

    nc = _build()
    maps = _in_maps(bert_x, x, ae, w)
    res = bass_utils.run_bass_kernel_spmd(
        nc, maps, core_ids=list(range(NCORES)), trace=trace
    )
    acc = np.concatenate(
        [
            res.results[k]["out"].astype(np.float32).reshape(BPC, L, H)
            for k in range(NCORES)
        ],
        axis=0,
    )
    out = np.asarray(bert_x, dtype=np.float32) + acc
    return out, res


def kernel(bert_x, x, ae, w):
    out, _ = _run(bert_x, x, ae, w, trace=False)
    return out


# revision 60
# speedup vs baseline: 1.0103x; 1.0103x over previous
"""Trainium2 Bass kernel for nn_CausalLayer (bilinear causal mixing layer).

Math (per batch b):
    E = ae[x]                                # [L, D] gather
    S[i,j] = E_i @ w @ E_j                   # bilinear pairwise score
    coef[i,j] = (i+1)/(j+1) for i<j else 0
    res[:,j] = bx[:,j] + sum_i coef[i,j]*S[i,j]*bx[:,i]

Chunked linear-attention identity (chunk C=128). With a_i = w^T E_i,
e'_j = E_j/(j+1) and y_i = (i+1)*bx_i:

    acc_j = M_cj @ e'_j + sum_{i<j, same chunk} ((i+1) a_i . e'_j) bx_i
    M_c   = sum_{i in chunks < c} y_i a_i^T      (rank-D running state, [D, H])
    res_j = bx_j + acc_j                          (final add on host)

Host prep (all O(L*D)): the ae gather, A = E @ w, the (i+1)/(1/(j+1)) row
and column scalings, and the [D, C] transposes. The device streams three
[*,768]-wide matmul groups plus one [128,128] score block per chunk — the
structural PE floor for this decomposition (~19 PE columns/token).

Schedule: per chunk the PE runs [M-update, S(next), out1, out2], the score
block software-pipelined one chunk ahead. The rank-D state M lives folded as
[128, 512] PSUM (H split 512/256 on partition halves, Et duplicated to
partitions 64-127 for the second half) so its bf16 snapshot is one cheap
Act-engine copy. The acc epilogue is two plain PSUM->bf16 copies split
DVE/Act. All aux-engine work fits inside the PE's chunk time even at the
2.4 GHz p-state, so the PE pipeline never starves. DMA moves 4 chunks per
descriptor batch, prefetched one group ahead; bf16 in and out with f32 PSUM
accumulation, ~5e-3 max scale-relative error vs the fp32 reference.

Sharding: batch-parallel, 2 of 16 batches per NeuronCore across 8 cores;
all tables are per-core slices. No cross-core communication.
"""

import os
import sys

for _p in ("/opt/trn_rl_repo", "/root/.axon_site/_ro/trn_rl_repo"):
    if os.path.isdir(_p) and _p not in sys.path:
        sys.path.insert(0, _p)

import numpy as np

B, L, H = 16, 2048, 768
V, D = 30000, 64
NCORES = 8
BPC = B // NCORES          # batches per core
C = 128                    # chunk (tile) size along sequence
NCH = L // C               # chunks per batch
ROWS = BPC * L             # bx rows per core
NT = BPC * NCH             # total chunks per core
G = 4                      # chunks per DMA group
HLO = 512                  # H split: [0:512] on partitions 0:64, [512:768] above
HHI = H - HLO

_compiled = {}


def _build():
    """Build + compile the per-core Bass module (SPMD: same program, 8 cores)."""
    key = "v4"
    if key in _compiled:
        return _compiled[key]

    import concourse.bacc as bacc
    import concourse.bass as bass
    import concourse.mybir as mybir
    import concourse.tile as tile

    f32 = mybir.dt.float32
    bf16 = mybir.dt.bfloat16

    nc = bacc.Bacc(
        "TRN2",
        target_bir_lowering=False,
        debug=False,
        enable_asserts=False,
        num_devices=NCORES,
    )

    bx_d = nc.dram_tensor("bx", [ROWS, H], bf16, kind="ExternalInput").ap()
    # per global chunk g: cols [g*2C, g*2C+C) = Et'_g ([D, C] transposed E,
    # column j scaled by 1/(j+1)), cols [g*2C+C, (g+1)*2C) = At'_g
    # ((i+1)-scaled transposed A)
    etat_d = nc.dram_tensor("etat", [D, 2 * ROWS], bf16, kind="ExternalInput").ap()
    # Ap rows aligned with bx rows: row i = (i+1) * a_i
    ap_d = nc.dram_tensor("ap", [ROWS, D], bf16, kind="ExternalInput").ap()
    um_d = nc.dram_tensor("umask", [C, C], f32, kind="ExternalInput").ap()
    out_d = nc.dram_tensor("out", [ROWS, H], bf16, kind="ExternalOutput").ap()

    mult = mybir.AluOpType.mult

    with tile.TileContext(nc) as tc:
        with (
            tc.tile_pool(name="const", bufs=1) as cpool,
            tc.tile_pool(name="bxp", bufs=4) as bxpool,
            tc.tile_pool(name="outp", bufs=3) as outpool,
            tc.tile_pool(name="eat", bufs=4) as eatpool,
            tc.tile_pool(name="app", bufs=4) as appool,
            tc.tile_pool(name="sm", bufs=4) as smpool,
            tc.tile_pool(name="mp", bufs=2) as mpool,
            tc.tile_pool(name="ps_s", bufs=2, space="PSUM") as ps_s,
            tc.tile_pool(name="ps_out", bufs=2, space="PSUM") as ps_out,
            tc.tile_pool(name="ps_m", bufs=2, space="PSUM") as ps_m,
        ):
            umask_s = cpool.tile([C, C], f32)

            bx_t = [None] * NT
            eat_t = [None] * NT
            ap_t = [None] * NT

            def load_group(gr, eng=None):
                """DMA one group of G chunks (bx / etat / ap).

                etat lands duplicated on both partition halves: the lower
                copy feeds the score block and out2-lo, the upper copy is
                the out2-hi stationary (PE array rows 64:128). Group 0 skips
                chunk 0, which the prologue fast-path loads separately so
                the pipeline starts without waiting on a full group."""
                eng = eng or nc.sync
                t0 = gr * G
                sk = 1 if gr == 0 else 0
                n = G - sk
                EAT4 = eatpool.tile(
                    [2 * D, G * 2 * C], bf16, name=f"EAT4_{gr}", tag="EAT4"
                )
                src = etat_d[:, (t0 + sk) * 2 * C:(t0 + G) * 2 * C]
                eng.dma_start(out=EAT4[0:D, sk * 2 * C:], in_=src)
                eng.dma_start(out=EAT4[D:2 * D, sk * 2 * C:], in_=src)
                AP4 = appool.tile([C, G * D], bf16, name=f"AP4_{gr}", tag="AP4")
                eng.dma_start(
                    out=AP4[:, sk * D:].rearrange("p (g d) -> p g d", g=n),
                    in_=ap_d[(t0 + sk) * C:(t0 + G) * C, :].rearrange(
                        "(g p) d -> p g d", g=n
                    ),
                )
                BX4 = bxpool.tile([C, G * H], bf16, name=f"BX4_{gr}", tag="BX4")
                eng.dma_start(
                    out=BX4[:, sk * H:].rearrange("p (g h) -> p g h", g=n),
                    in_=bx_d[(t0 + sk) * C:(t0 + G) * C, :].rearrange(
                        "(g p) h -> p g h", g=n
                    ),
                )
                for q in range(sk, G):
                    t = t0 + q
                    bx_t[t] = BX4[:, q * H:(q + 1) * H]
                    eat_t[t] = EAT4
                    ap_t[t] = AP4[:, q * D:(q + 1) * D]

            def s_block(t):
                """Score block S'(t) on PE + mask on DVE (pipelined ahead)."""
                q = t % G
                Atp = eat_t[t][0:D, q * 2 * C + C:(q + 1) * 2 * C]
                Etp = eat_t[t][0:D, q * 2 * C:q * 2 * C + C]
                s_p = ps_s.tile([C, C], f32, name=f"s_p_{t}", tag="s_p")
                nc.tensor.matmul(
                    out=s_p[:], lhsT=Atp, rhs=Etp, start=True, stop=True,
                )
                St = smpool.tile([C, C], bf16, name=f"St_{t}", tag="St")
                nc.vector.tensor_tensor(
                    out=St[:], in0=s_p[:], in1=umask_s[:], op=mult,
                )
                return St

            # fast-path chunk 0: tiny dedicated loads so the score block and
            # the first out1/M-update start as early as possible; the posts
            # are spread across queues so they configure DGE concurrently
            # instead of serializing on the Sync sequencer
            # (chunk 0 has no out2, so the upper-partition Et duplicate is
            # never read from this tile — one load is enough)
            eat0 = cpool.tile([2 * D, 2 * C], bf16)
            nc.scalar.dma_start(out=eat0[0:D, :], in_=etat_d[:, 0:2 * C])
            bx0 = cpool.tile([C, H], bf16)
            nc.sync.dma_start(out=bx0[:], in_=bx_d[0:C, :])
            ap0 = cpool.tile([C, D], bf16)
            nc.gpsimd.dma_start(out=ap0[:], in_=ap_d[0:C, :])
            nc.sync.dma_start(out=umask_s[:], in_=um_d[:, :])
            bx_t[0] = bx0[:, :]
            eat_t[0] = eat0
            ap_t[0] = ap0[:, :]

            St_next = s_block(0)
            load_group(0)
            load_group(1)

            M_p = None
            M_s = None
            for t in range(NT):
                b, c = divmod(t, NCH)
                q = t % G
                BX = bx_t[t]
                Etp = eat_t[t][0:D, q * 2 * C:q * 2 * C + C]
                EtpD = eat_t[t][D:2 * D, q * 2 * C:q * 2 * C + C]

                if q == 0 and t // G + 2 < NT // G:
                    load_group(t // G + 2)

                if c == 0:
                    # folded rank-D state: partitions 0:64 hold M[:, 0:512],
                    # partitions 64:128 hold M[:, 512:768] (cols 256:512 of
                    # the upper half are dead; zero them once so the bf16
                    # snapshot below never reads uninitialized PSUM)
                    M_p = ps_m.tile([2 * D, HLO], f32, name=f"M_p_b{b}", tag="M_p")
                    nc.vector.memset(M_p[D:2 * D, HHI:HLO], 0.0)

                # M += y^T-outer-a, folded  (skip the never-read last update).
                # skip_group_check: the sim's group guard can't express this
                # read-between-accumulations pattern; the pending-zero
                # accumulate semantics and Tile's HW sync are unaffected.
                if c < NCH - 1:
                    nc.tensor.matmul(
                        out=M_p[0:D, 0:HLO],
                        lhsT=ap_t[t],
                        rhs=BX[:, 0:HLO],
                        start=(c == 0),
                        stop=True,
                        skip_group_check=True,
                    )
                    nc.tensor.matmul(
                        out=M_p[D:2 * D, 0:HHI],
                        lhsT=ap_t[t],
                        rhs=BX[:, HLO:H],
                        start=(c == 0),
                        stop=True,
                        skip_group_check=True,
                    )

                St = St_next
                if t + 1 < NT:
                    St_next = s_block(t + 1)

                # acc = St^T @ BX (+ Et'^T @ M)  [C, H]
                out_p = ps_out.tile([C, H], f32, name=f"out_p_{t}", tag="out_p")
                for lo, hi in ((0, HLO), (HLO, H)):
                    nc.tensor.matmul(
                        out=out_p[:, lo:hi],
                        lhsT=St[:],
                        rhs=BX[:, lo:hi],
                        start=True,
                        stop=(c == 0),
                    )
                if c > 0:
                    nc.tensor.matmul(
                        out=out_p[:, 0:HLO],
                        lhsT=Etp,
                        rhs=M_s[0:D, 0:HLO],
                        start=False,
                        stop=True,
                    )
                    nc.tensor.matmul(
                        out=out_p[:, HLO:H],
                        lhsT=EtpD,
                        rhs=M_s[D:2 * D, 0:HHI],
                        start=False,
                        stop=True,
                    )

                # snapshot M for the NEXT chunk (reads M_p after this chunk's
                # update, before the next one; the Act engine runs it as soon
                # as the update's semaphore fires, independent of issue order)
                if t + 1 < NT and (t + 1) % NCH != 0:
                    M_s = mpool.tile([2 * D, HLO], bf16, name=f"M_s_{t + 1}", tag="M_s")
                    nc.scalar.copy(out=M_s[:], in_=M_p[:])

                # acc -> bf16 out tile on DVE (the Act engine stays dedicated
                # to M snapshots so its queue never backs up behind out_p);
                # grouped stores ride the otherwise-idle GpSimd queue so the
                # Sync queue's load posts never block behind compute
                if q == 0:
                    OUT4 = outpool.tile([C, G * H], bf16, name=f"OUT4_{t}", tag="OUT4")
                nc.vector.tensor_scalar_add(
                    out=OUT4[:, q * H:(q + 1) * H],
                    in0=out_p[:],
                    scalar1=0.0,
                )
                if t >= NT - G:
                    # final group: store per chunk so the drain after the
                    # last acc copy is one small transfer, not 4 chunks
                    nc.gpsimd.dma_start(
                        out=out_d[t * C:(t + 1) * C, :],
                        in_=OUT4[:, q * H:(q + 1) * H],
                    )
                elif q == G - 1:
                    t0 = t - G + 1
                    nc.gpsimd.dma_start(
                        out=out_d[t0 * C:(t + 1) * C, :].rearrange(
                            "(g p) h -> p g h", g=G
                        ),
                        in_=OUT4[:].rearrange("p (g h) -> p g h", g=G),
                    )

    # Adjacent PE matmuls often share a stationary operand (the two H-halves
    # of out1); legalization has already paired each matmul with a standalone
    # InstLdweights, so drop the redundant reloads. The key includes the PE
    # array tile position: the same weights loaded into a different array
    # quadrant is a genuine reload.
    ndropped = 0
    for blk in nc.m.functions[0].blocks:
        keep = []
        last_w = None
        for inst in blk.instructions:
            if getattr(inst, "engine", None) == mybir.EngineType.PE:
                if isinstance(inst, mybir.InstLdweights):
                    w = inst.ins[0]
                    wkey = (
                        w.memref,
                        w.offset,
                        str(w.ap),
                        str(getattr(inst, "tile_position", None)),
                        str(getattr(inst, "tile_size", None)),
                    )
                    if (
                        last_w is not None
                        and wkey == last_w
                        and not inst.has_wait()
                    ):
                        ndropped += 1
                        continue
                    last_w = wkey
                elif not isinstance(inst, mybir.InstMatmult):
                    last_w = None
            keep.append(inst)
        blk.instructions = keep
    if os.environ.get("BASS_DEBUG_FUSE"):
        print(f"[kernel] redundant ldweights dropped: {ndropped}", file=sys.stderr)

    nc.compile()
    _compiled[key] = nc
    return nc


def _np_umask():
    i = np.arange(C)
    return (i[:, None] < i[None, :]).astype(np.float32)


def _in_maps(bert_x, x, ae, w):
    import ml_dtypes

    bf16 = ml_dtypes.bfloat16
    bert_x = np.asarray(bert_x, dtype=np.float32)
    x = np.asarray(x)
    ae = np.asarray(ae, dtype=np.float32)
    w = np.asarray(w, dtype=np.float32)

    E = ae[x.reshape(-1)]                     # [B*L, D]
    A = E @ w                                 # [B*L, D]
    jp1 = (np.arange(L, dtype=np.float64) + 1.0).astype(np.float32)
    Ap = (A.reshape(B, L, D) * jp1[None, :, None]).reshape(B * L, D)
    Einv = (E.reshape(B, L, D) / jp1[None, :, None]).reshape(B * L, D)

    bx16 = np.ascontiguousarray(bert_x.reshape(B * L, H).astype(bf16))
    ap16 = np.ascontiguousarray(Ap.astype(bf16))

    # etat per core: [D, 2*ROWS]; per global chunk g: [Et'_g | At'_g]
    Ech = Einv.reshape(B, NCH, C, D).astype(bf16)
    Ach = Ap.reshape(B, NCH, C, D).astype(bf16)
    pair = np.stack([Ech, Ach], axis=2)       # [B, NCH, 2, C, D]
    pair = pair.transpose(0, 4, 1, 2, 3)      # [B, D, NCH, 2, C]

    umask = _np_umask()
    maps = []
    for k in range(NCORES):
        et = np.ascontiguousarray(
            pair[k * BPC:(k + 1) * BPC].transpose(1, 0, 2, 3, 4).reshape(D, 2 * ROWS)
        )
        maps.append(
            {
                "bx": bx16[k * BPC * L:(k + 1) * BPC * L],
                "etat": et,
                "ap": ap16[k * BPC * L:(k + 1) * BPC * L],
                "umask": umask,
            }
        )
    return maps


def _run(bert_x, x, ae, w, trace=False):
    from concourse import bass_utils

    nc = _build()
    maps = _in_maps(bert_x, x, ae, w)
    res = bass_utils.run_bass_kernel_spmd(
        nc, maps, core_ids=list(range(NCORES)), trace=trace
    )
    acc = np.concatenate(
        [
            res.results[k]["out"].astype(np.float32).reshape(BPC, L, H)
            for k in range(NCORES)
        ],
        axis=0,
    )
    out = np.asarray(bert_x, dtype=np.float32) + acc
    return out, res


def kernel(bert_x, x, ae, w):
    out, _ = _run(bert_x, x, ae, w, trace=False)
    return out


# revision 61
# speedup vs baseline: 1.0518x; 1.0412x over previous
"""Trainium2 Bass kernel for nn_CausalLayer (bilinear causal mixing layer).

Math (per batch b):
    E = ae[x]                                # [L, D] gather
    S[i,j] = E_i @ w @ E_j                   # bilinear pairwise score
    coef[i,j] = (i+1)/(j+1) for i<j else 0
    res[:,j] = bx[:,j] + sum_i coef[i,j]*S[i,j]*bx[:,i]

Chunked linear-attention identity (chunk C=128). With a_i = w^T E_i,
e'_j = E_j/(j+1) and y_i = (i+1)*bx_i:

    acc_j = M_cj @ e'_j + sum_{i<j, same chunk} ((i+1) a_i . e'_j) bx_i
    M_c   = sum_{i in chunks < c} y_i a_i^T      (rank-D running state, [D, H])
    res_j = bx_j + acc_j                          (final add on host)

Host prep (all O(L*D)): the ae gather, A = E @ w, the (i+1)/(1/(j+1)) row
and column scalings, and the [D, C] transposes. The device streams three
[*,768]-wide matmul groups plus one [128,128] score block per chunk — the
structural PE floor for this decomposition (~19 PE columns/token).

Schedule: per chunk the PE runs [M-update, S(next), out1, out2], the score
block software-pipelined one chunk ahead. The rank-D state M lives folded as
[128, 512] PSUM (H split 512/256 on partition halves, Et duplicated to
partitions 64-127 for the second half) so its bf16 snapshot is one cheap
Act-engine copy. The acc epilogue is two plain PSUM->bf16 copies split
DVE/Act. All aux-engine work fits inside the PE's chunk time even at the
2.4 GHz p-state, so the PE pipeline never starves. DMA moves 4 chunks per
descriptor batch, prefetched one group ahead; bf16 in and out with f32 PSUM
accumulation, ~5e-3 max scale-relative error vs the fp32 reference.

Sharding: batch-parallel, 2 of 16 batches per NeuronCore across 8 cores;
all tables are per-core slices. No cross-core communication.
"""

import os
import sys

for _p in ("/opt/trn_rl_repo", "/root/.axon_site/_ro/trn_rl_repo"):
    if os.path.isdir(_p) and _p not in sys.path:
        sys.path.insert(0, _p)

import numpy as np

B, L, H = 16, 2048, 768
V, D = 30000, 64
NCORES = 8
BPC = B // NCORES          # batches per core
C = 128                    # chunk (tile) size along sequence
NCH = L // C               # chunks per batch
ROWS = BPC * L             # bx rows per core
NT = BPC * NCH             # total chunks per core
G = 4                      # chunks per DMA group
HLO = 512                  # H split: [0:512] on partitions 0:64, [512:768] above
HHI = H - HLO

_compiled = {}


def _build():
    """Build + compile the per-core Bass module (SPMD: same program, 8 cores)."""
    key = "v4"
    if key in _compiled:
        return _compiled[key]

    import concourse.bacc as bacc
    import concourse.bass as bass
    import concourse.mybir as mybir
    import concourse.tile as tile

    f32 = mybir.dt.float32
    bf16 = mybir.dt.bfloat16

    nc = bacc.Bacc(
        "TRN2",
        target_bir_lowering=False,
        debug=False,
        enable_asserts=False,
        num_devices=NCORES,
    )

    bx_d = nc.dram_tensor("bx", [ROWS, H], bf16, kind="ExternalInput").ap()
    # per global chunk g: cols [g*2C, g*2C+C) = Et'_g ([D, C] transposed E,
    # column j scaled by 1/(j+1)), cols [g*2C+C, (g+1)*2C) = At'_g
    # ((i+1)-scaled transposed A)
    etat_d = nc.dram_tensor("etat", [D, 2 * ROWS], bf16, kind="ExternalInput").ap()
    # Ap rows aligned with bx rows: row i = (i+1) * a_i
    ap_d = nc.dram_tensor("ap", [ROWS, D], bf16, kind="ExternalInput").ap()
    um_d = nc.dram_tensor("umask", [C, C], f32, kind="ExternalInput").ap()
    out_d = nc.dram_tensor("out", [ROWS, H], bf16, kind="ExternalOutput").ap()

    mult = mybir.AluOpType.mult

    with tile.TileContext(nc) as tc:
        with (
            tc.tile_pool(name="const", bufs=1) as cpool,
            tc.tile_pool(name="bxp", bufs=4) as bxpool,
            tc.tile_pool(name="outp", bufs=3) as outpool,
            tc.tile_pool(name="eat", bufs=4) as eatpool,
            tc.tile_pool(name="app", bufs=4) as appool,
            tc.tile_pool(name="sm", bufs=6) as smpool,
            tc.tile_pool(name="mp", bufs=3) as mpool,
            tc.tile_pool(name="ps_s", bufs=2, space="PSUM") as ps_s,
            tc.tile_pool(name="ps_out", bufs=2, space="PSUM") as ps_out,
            tc.tile_pool(name="ps_m", bufs=2, space="PSUM") as ps_m,
        ):
            umask_s = cpool.tile([C, C], f32)

            bx_t = [None] * NT
            eat_t = [None] * NT
            ap_t = [None] * NT

            def load_group(gr, eng=None):
                """DMA one group of G chunks (bx / etat / ap).

                etat lands duplicated on both partition halves: the lower
                copy feeds the score block and out2-lo, the upper copy is
                the out2-hi stationary (PE array rows 64:128). Group 0 skips
                chunk 0, which the prologue fast-path loads separately so
                the pipeline starts without waiting on a full group."""
                eng = eng or nc.sync
                t0 = gr * G
                sk = 1 if gr == 0 else 0
                n = G - sk
                EAT4 = eatpool.tile(
                    [2 * D, G * 2 * C], bf16, name=f"EAT4_{gr}", tag="EAT4"
                )
                src = etat_d[:, (t0 + sk) * 2 * C:(t0 + G) * 2 * C]
                eng.dma_start(out=EAT4[0:D, sk * 2 * C:], in_=src)
                eng.dma_start(out=EAT4[D:2 * D, sk * 2 * C:], in_=src)
                AP4 = appool.tile([C, G * D], bf16, name=f"AP4_{gr}", tag="AP4")
                eng.dma_start(
                    out=AP4[:, sk * D:].rearrange("p (g d) -> p g d", g=n),
                    in_=ap_d[(t0 + sk) * C:(t0 + G) * C, :].rearrange(
                        "(g p) d -> p g d", g=n
                    ),
                )
                BX4 = bxpool.tile([C, G * H], bf16, name=f"BX4_{gr}", tag="BX4")
                eng.dma_start(
                    out=BX4[:, sk * H:].rearrange("p (g h) -> p g h", g=n),
                    in_=bx_d[(t0 + sk) * C:(t0 + G) * C, :].rearrange(
                        "(g p) h -> p g h", g=n
                    ),
                )
                for q in range(sk, G):
                    t = t0 + q
                    bx_t[t] = BX4[:, q * H:(q + 1) * H]
                    eat_t[t] = EAT4
                    ap_t[t] = AP4[:, q * D:(q + 1) * D]

            def s_block(t):
                """Score block S'(t) on PE + mask on DVE (pipelined ahead)."""
                q = t % G
                Atp = eat_t[t][0:D, q * 2 * C + C:(q + 1) * 2 * C]
                Etp = eat_t[t][0:D, q * 2 * C:q * 2 * C + C]
                s_p = ps_s.tile([C, C], f32, name=f"s_p_{t}", tag="s_p")
                nc.tensor.matmul(
                    out=s_p[:], lhsT=Atp, rhs=Etp, start=True, stop=True,
                )
                St = smpool.tile([C, C], bf16, name=f"St_{t}", tag="St")
                nc.vector.tensor_tensor(
                    out=St[:], in0=s_p[:], in1=umask_s[:], op=mult,
                )
                return St

            # fast-path chunk 0: tiny dedicated loads so the score block and
            # the first out1/M-update start as early as possible; the posts
            # are spread across queues so they configure DGE concurrently
            # instead of serializing on the Sync sequencer
            # (chunk 0 has no out2, so the upper-partition Et duplicate is
            # never read from this tile — one load is enough)
            eat0 = cpool.tile([2 * D, 2 * C], bf16)
            nc.scalar.dma_start(out=eat0[0:D, :], in_=etat_d[:, 0:2 * C])
            bx0 = cpool.tile([C, H], bf16)
            nc.sync.dma_start(out=bx0[:], in_=bx_d[0:C, :])
            ap0 = cpool.tile([C, D], bf16)
            nc.gpsimd.dma_start(out=ap0[:], in_=ap_d[0:C, :])
            nc.sync.dma_start(out=umask_s[:], in_=um_d[:, :])
            bx_t[0] = bx0[:, :]
            eat_t[0] = eat0
            ap_t[0] = ap0[:, :]

            St_next = s_block(0)
            load_group(0)
            load_group(1)

            M_p = None
            M_s = None
            for t in range(NT):
                b, c = divmod(t, NCH)
                q = t % G
                BX = bx_t[t]
                Etp = eat_t[t][0:D, q * 2 * C:q * 2 * C + C]
                EtpD = eat_t[t][D:2 * D, q * 2 * C:q * 2 * C + C]

                if q == 0 and t // G + 2 < NT // G:
                    load_group(t // G + 2)

                if c == 0:
                    # folded rank-D state: partitions 0:64 hold M[:, 0:512],
                    # partitions 64:128 hold M[:, 512:768] (cols 256:512 of
                    # the upper half are dead; zero them once so the bf16
                    # snapshot below never reads uninitialized PSUM)
                    M_p = ps_m.tile([2 * D, HLO], f32, name=f"M_p_b{b}", tag="M_p")
                    nc.vector.memset(M_p[D:2 * D, HHI:HLO], 0.0)

                # M += y^T-outer-a, folded  (skip the never-read last update).
                # skip_group_check: the sim's group guard can't express this
                # read-between-accumulations pattern; the pending-zero
                # accumulate semantics and Tile's HW sync are unaffected.
                if c < NCH - 1:
                    nc.tensor.matmul(
                        out=M_p[0:D, 0:HLO],
                        lhsT=ap_t[t],
                        rhs=BX[:, 0:HLO],
                        start=(c == 0),
                        stop=True,
                        skip_group_check=True,
                    )
                    nc.tensor.matmul(
                        out=M_p[D:2 * D, 0:HHI],
                        lhsT=ap_t[t],
                        rhs=BX[:, HLO:H],
                        start=(c == 0),
                        stop=True,
                        skip_group_check=True,
                    )

                St = St_next
                if t + 1 < NT:
                    St_next = s_block(t + 1)

                # acc = St^T @ BX (+ Et'^T @ M)  [C, H]
                out_p = ps_out.tile([C, H], f32, name=f"out_p_{t}", tag="out_p")
                for lo, hi in ((0, HLO), (HLO, H)):
                    nc.tensor.matmul(
                        out=out_p[:, lo:hi],
                        lhsT=St[:],
                        rhs=BX[:, lo:hi],
                        start=True,
                        stop=(c == 0),
                    )
                if c > 0:
                    nc.tensor.matmul(
                        out=out_p[:, 0:HLO],
                        lhsT=Etp,
                        rhs=M_s[0:D, 0:HLO],
                        start=False,
                        stop=True,
                    )
                    nc.tensor.matmul(
                        out=out_p[:, HLO:H],
                        lhsT=EtpD,
                        rhs=M_s[D:2 * D, 0:HHI],
                        start=False,
                        stop=True,
                    )

                # snapshot M for the NEXT chunk (reads M_p after this chunk's
                # update, before the next one; the Act engine runs it as soon
                # as the update's semaphore fires, independent of issue order)
                if t + 1 < NT and (t + 1) % NCH != 0:
                    M_s = mpool.tile([2 * D, HLO], bf16, name=f"M_s_{t + 1}", tag="M_s")
                    nc.scalar.copy(out=M_s[:], in_=M_p[:])

                # acc -> bf16 out tile on DVE (the Act engine stays dedicated
                # to M snapshots so its queue never backs up behind out_p);
                # grouped stores ride the otherwise-idle GpSimd queue so the
                # Sync queue's load posts never block behind compute
                if q == 0:
                    OUT4 = outpool.tile([C, G * H], bf16, name=f"OUT4_{t}", tag="OUT4")
                nc.vector.tensor_scalar_add(
                    out=OUT4[:, q * H:(q + 1) * H],
                    in0=out_p[:],
                    scalar1=0.0,
                )
                if t >= NT - G:
                    # final group: store per chunk so the drain after the
                    # last acc copy is one small transfer, not 4 chunks
                    nc.gpsimd.dma_start(
                        out=out_d[t * C:(t + 1) * C, :],
                        in_=OUT4[:, q * H:(q + 1) * H],
                    )
                elif q == G - 1:
                    t0 = t - G + 1
                    nc.gpsimd.dma_start(
                        out=out_d[t0 * C:(t + 1) * C, :].rearrange(
                            "(g p) h -> p g h", g=G
                        ),
                        in_=OUT4[:].rearrange("p (g h) -> p g h", g=G),
                    )

    # Adjacent PE matmuls often share a stationary operand (the two H-halves
    # of out1); legalization has already paired each matmul with a standalone
    # InstLdweights, so drop the redundant reloads. The key includes the PE
    # array tile position: the same weights loaded into a different array
    # quadrant is a genuine reload.
    ndropped = 0
    for blk in nc.m.functions[0].blocks:
        keep = []
        last_w = None
        for inst in blk.instructions:
            if getattr(inst, "engine", None) == mybir.EngineType.PE:
                if isinstance(inst, mybir.InstLdweights):
                    w = inst.ins[0]
                    wkey = (
                        w.memref,
                        w.offset,
                        str(w.ap),
                        str(getattr(inst, "tile_position", None)),
                        str(getattr(inst, "tile_size", None)),
                    )
                    if (
                        last_w is not None
                        and wkey == last_w
                        and not inst.has_wait()
                    ):
                        ndropped += 1
                        continue
                    last_w = wkey
                elif not isinstance(inst, mybir.InstMatmult):
                    last_w = None
            keep.append(inst)
        blk.instructions = keep
    if os.environ.get("BASS_DEBUG_FUSE"):
        print(f"[kernel] redundant ldweights dropped: {ndropped}", file=sys.stderr)

    nc.compile()
    _compiled[key] = nc
    return nc


def _np_umask():
    i = np.arange(C)
    return (i[:, None] < i[None, :]).astype(np.float32)


def _in_maps(bert_x, x, ae, w):
    import ml_dtypes

    bf16 = ml_dtypes.bfloat16
    bert_x = np.asarray(bert_x, dtype=np.float32)
    x = np.asarray(x)
    ae = np.asarray(ae, dtype=np.float32)
    w = np.asarray(w, dtype=np.float32)

    E = ae[x.reshape(-1)]                     # [B*L, D]
    A = E @ w                                 # [B*L, D]
    jp1 = (np.arange(L, dtype=np.float64) + 1.0).astype(np.float32)
    Ap = (A.reshape(B, L, D) * jp1[None, :, None]).reshape(B * L, D)
    Einv = (E.reshape(B, L, D) / jp1[None, :, None]).reshape(B * L, D)

    bx16 = np.ascontiguousarray(bert_x.reshape(B * L, H).astype(bf16))
    ap16 = np.ascontiguousarray(Ap.astype(bf16))

    # etat per core: [D, 2*ROWS]; per global chunk g: [Et'_g | At'_g]
    Ech = Einv.reshape(B, NCH, C, D).astype(bf16)
    Ach = Ap.reshape(B, NCH, C, D).astype(bf16)
    pair = np.stack([Ech, Ach], axis=2)       # [B, NCH, 2, C, D]
    pair = pair.transpose(0, 4, 1, 2, 3)      # [B, D, NCH, 2, C]

    umask = _np_umask()
    maps = []
    for k in range(NCORES):
        et = np.ascontiguousarray(
            pair[k * BPC:(k + 1) * BPC].transpose(1, 0, 2, 3, 4).reshape(D, 2 * ROWS)
        )
        maps.append(
            {
                "bx": bx16[k * BPC * L:(k + 1) * BPC * L],
                "etat": et,
                "ap": ap16[k * BPC * L:(k + 1) * BPC * L],
                "umask": umask,
            }
        )
    return maps


def _run(bert_x, x, ae, w, trace=False):
    from concourse import bass_utils

    nc = _build()
    maps = _in_maps(bert_x, x, ae, w)
    res = bass_utils.run_bass_kernel_spmd(
        nc, maps, core_ids=list(range(NCORES)), trace=trace
    )
    acc = np.concatenate(
        [
            res.results[k]["out"].astype(np.float32).reshape(BPC, L, H)
            for k in range(NCORES)
        ],
        axis=0,
    )
    out = np.asarray(bert_x, dtype=np.float32) + acc
    return out, res


def kernel(bert_x, x, ae, w):
    out, _ = _run(bert_x, x, ae, w, trace=False)
    return out


# revision 62
# speedup vs baseline: 1.0537x; 1.0018x over previous
"""Trainium2 Bass kernel for nn_CausalLayer (bilinear causal mixing layer).

Math (per batch b):
    E = ae[x]                                # [L, D] gather
    S[i,j] = E_i @ w @ E_j                   # bilinear pairwise score
    coef[i,j] = (i+1)/(j+1) for i<j else 0
    res[:,j] = bx[:,j] + sum_i coef[i,j]*S[i,j]*bx[:,i]

Chunked linear-attention identity (chunk C=128). With a_i = w^T E_i,
e'_j = E_j/(j+1) and y_i = (i+1)*bx_i:

    acc_j = M_cj @ e'_j + sum_{i<j, same chunk} ((i+1) a_i . e'_j) bx_i
    M_c   = sum_{i in chunks < c} y_i a_i^T      (rank-D running state, [D, H])
    res_j = bx_j + acc_j                          (final add on host)

Host prep (all O(L*D)): the ae gather, A = E @ w, the (i+1)/(1/(j+1)) row
and column scalings, and the [D, C] transposes. The device streams three
[*,768]-wide matmul groups plus one [128,128] score block per chunk — the
structural PE floor for this decomposition (~19 PE columns/token).

Schedule: per chunk the PE runs [M-update, S(next), out1, out2], the score
block software-pipelined one chunk ahead. The rank-D state M lives folded as
[128, 512] PSUM (H split 512/256 on partition halves, Et duplicated to
partitions 64-127 for the second half) so its bf16 snapshot is one cheap
Act-engine copy. The acc epilogue is two plain PSUM->bf16 copies split
DVE/Act. All aux-engine work fits inside the PE's chunk time even at the
2.4 GHz p-state, so the PE pipeline never starves. DMA moves 4 chunks per
descriptor batch, prefetched one group ahead; bf16 in and out with f32 PSUM
accumulation, ~5e-3 max scale-relative error vs the fp32 reference.

Sharding: batch-parallel, 2 of 16 batches per NeuronCore across 8 cores;
all tables are per-core slices. No cross-core communication.
"""

import os
import sys

for _p in ("/opt/trn_rl_repo", "/root/.axon_site/_ro/trn_rl_repo"):
    if os.path.isdir(_p) and _p not in sys.path:
        sys.path.insert(0, _p)

import numpy as np

B, L, H = 16, 2048, 768
V, D = 30000, 64
NCORES = 8
BPC = B // NCORES          # batches per core
C = 128                    # chunk (tile) size along sequence
NCH = L // C               # chunks per batch
ROWS = BPC * L             # bx rows per core
NT = BPC * NCH             # total chunks per core
G = 4                      # chunks per DMA group
HLO = 512                  # H split: [0:512] on partitions 0:64, [512:768] above
HHI = H - HLO

_compiled = {}


def _build():
    """Build + compile the per-core Bass module (SPMD: same program, 8 cores)."""
    key = "v4"
    if key in _compiled:
        return _compiled[key]

    import concourse.bacc as bacc
    import concourse.bass as bass
    import concourse.mybir as mybir
    import concourse.tile as tile

    f32 = mybir.dt.float32
    bf16 = mybir.dt.bfloat16

    nc = bacc.Bacc(
        "TRN2",
        target_bir_lowering=False,
        debug=False,
        enable_asserts=False,
        num_devices=NCORES,
    )

    bx_d = nc.dram_tensor("bx", [ROWS, H], bf16, kind="ExternalInput").ap()
    # per global chunk g: cols [g*2C, g*2C+C) = Et'_g ([D, C] transposed E,
    # column j scaled by 1/(j+1)), cols [g*2C+C, (g+1)*2C) = At'_g
    # ((i+1)-scaled transposed A)
    etat_d = nc.dram_tensor("etat", [D, 2 * ROWS], bf16, kind="ExternalInput").ap()
    # Ap rows aligned with bx rows: row i = (i+1) * a_i
    ap_d = nc.dram_tensor("ap", [ROWS, D], bf16, kind="ExternalInput").ap()
    um_d = nc.dram_tensor("umask", [C, C], f32, kind="ExternalInput").ap()
    out_d = nc.dram_tensor("out", [ROWS, H], bf16, kind="ExternalOutput").ap()

    mult = mybir.AluOpType.mult

    with tile.TileContext(nc) as tc:
        with (
            tc.tile_pool(name="const", bufs=1) as cpool,
            tc.tile_pool(name="bxp", bufs=4) as bxpool,
            tc.tile_pool(name="outp", bufs=3) as outpool,
            tc.tile_pool(name="eat", bufs=4) as eatpool,
            tc.tile_pool(name="app", bufs=4) as appool,
            tc.tile_pool(name="sm", bufs=4) as smpool,
            tc.tile_pool(name="mp", bufs=2) as mpool,
            tc.tile_pool(name="ps_s", bufs=2, space="PSUM") as ps_s,
            tc.tile_pool(name="ps_out", bufs=2, space="PSUM") as ps_out,
            tc.tile_pool(name="ps_m", bufs=2, space="PSUM") as ps_m,
        ):
            umask_s = cpool.tile([C, C], f32)

            bx_t = [None] * NT
            eat_t = [None] * NT
            ap_t = [None] * NT

            def load_group(gr, eng=None):
                """DMA one group of G chunks (bx / etat / ap).

                etat lands duplicated on both partition halves: the lower
                copy feeds the score block and out2-lo, the upper copy is
                the out2-hi stationary (PE array rows 64:128). Group 0 skips
                chunk 0, which the prologue fast-path loads separately so
                the pipeline starts without waiting on a full group."""
                eng = eng or nc.sync
                t0 = gr * G
                sk = 1 if gr == 0 else 0
                n = G - sk
                EAT4 = eatpool.tile(
                    [2 * D, G * 2 * C], bf16, name=f"EAT4_{gr}", tag="EAT4"
                )
                src = etat_d[:, (t0 + sk) * 2 * C:(t0 + G) * 2 * C]
                eng.dma_start(out=EAT4[0:D, sk * 2 * C:], in_=src)
                eng.dma_start(out=EAT4[D:2 * D, sk * 2 * C:], in_=src)
                AP4 = appool.tile([C, G * D], bf16, name=f"AP4_{gr}", tag="AP4")
                eng.dma_start(
                    out=AP4[:, sk * D:].rearrange("p (g d) -> p g d", g=n),
                    in_=ap_d[(t0 + sk) * C:(t0 + G) * C, :].rearrange(
                        "(g p) d -> p g d", g=n
                    ),
                )
                BX4 = bxpool.tile([C, G * H], bf16, name=f"BX4_{gr}", tag="BX4")
                eng.dma_start(
                    out=BX4[:, sk * H:].rearrange("p (g h) -> p g h", g=n),
                    in_=bx_d[(t0 + sk) * C:(t0 + G) * C, :].rearrange(
                        "(g p) h -> p g h", g=n
                    ),
                )
                for q in range(sk, G):
                    t = t0 + q
                    bx_t[t] = BX4[:, q * H:(q + 1) * H]
                    eat_t[t] = EAT4
                    ap_t[t] = AP4[:, q * D:(q + 1) * D]

            def s_block(t):
                """Score block S'(t) on PE + mask on DVE (pipelined ahead)."""
                q = t % G
                Atp = eat_t[t][0:D, q * 2 * C + C:(q + 1) * 2 * C]
                Etp = eat_t[t][0:D, q * 2 * C:q * 2 * C + C]
                s_p = ps_s.tile([C, C], f32, name=f"s_p_{t}", tag="s_p")
                nc.tensor.matmul(
                    out=s_p[:], lhsT=Atp, rhs=Etp, start=True, stop=True,
                )
                St = smpool.tile([C, C], bf16, name=f"St_{t}", tag="St")
                nc.vector.tensor_tensor(
                    out=St[:], in0=s_p[:], in1=umask_s[:], op=mult,
                )
                return St

            # fast-path chunk 0: tiny dedicated loads so the score block and
            # the first out1/M-update start as early as possible; the posts
            # are spread across queues so they configure DGE concurrently
            # instead of serializing on the Sync sequencer
            # (chunk 0 has no out2, so the upper-partition Et duplicate is
            # never read from this tile — one load is enough)
            eat0 = cpool.tile([2 * D, 2 * C], bf16)
            nc.scalar.dma_start(out=eat0[0:D, :], in_=etat_d[:, 0:2 * C])
            bx0 = cpool.tile([C, H], bf16)
            nc.sync.dma_start(out=bx0[:], in_=bx_d[0:C, :])
            ap0 = cpool.tile([C, D], bf16)
            nc.gpsimd.dma_start(out=ap0[:], in_=ap_d[0:C, :])
            nc.sync.dma_start(out=umask_s[:], in_=um_d[:, :])
            bx_t[0] = bx0[:, :]
            eat_t[0] = eat0
            ap_t[0] = ap0[:, :]

            St_next = s_block(0)
            load_group(0)
            load_group(1)

            M_p = None
            M_s = None
            for t in range(NT):
                b, c = divmod(t, NCH)
                q = t % G
                BX = bx_t[t]
                Etp = eat_t[t][0:D, q * 2 * C:q * 2 * C + C]
                EtpD = eat_t[t][D:2 * D, q * 2 * C:q * 2 * C + C]

                if q == 0 and t // G + 2 < NT // G:
                    load_group(t // G + 2)

                if c == 0:
                    # folded rank-D state: partitions 0:64 hold M[:, 0:512],
                    # partitions 64:128 hold M[:, 512:768] (cols 256:512 of
                    # the upper half are dead; zero them once so the bf16
                    # snapshot below never reads uninitialized PSUM)
                    M_p = ps_m.tile([2 * D, HLO], f32, name=f"M_p_b{b}", tag="M_p")
                    nc.vector.memset(M_p[D:2 * D, HHI:HLO], 0.0)

                # M += y^T-outer-a, folded  (skip the never-read last update).
                # skip_group_check: the sim's group guard can't express this
                # read-between-accumulations pattern; the pending-zero
                # accumulate semantics and Tile's HW sync are unaffected.
                if c < NCH - 1:
                    nc.tensor.matmul(
                        out=M_p[0:D, 0:HLO],
                        lhsT=ap_t[t],
                        rhs=BX[:, 0:HLO],
                        start=(c == 0),
                        stop=True,
                        skip_group_check=True,
                    )
                    nc.tensor.matmul(
                        out=M_p[D:2 * D, 0:HHI],
                        lhsT=ap_t[t],
                        rhs=BX[:, HLO:H],
                        start=(c == 0),
                        stop=True,
                        skip_group_check=True,
                    )

                St = St_next
                if t + 1 < NT:
                    St_next = s_block(t + 1)

                # acc = St^T @ BX (+ Et'^T @ M)  [C, H]
                out_p = ps_out.tile([C, H], f32, name=f"out_p_{t}", tag="out_p")
                for lo, hi in ((0, HLO), (HLO, H)):
                    nc.tensor.matmul(
                        out=out_p[:, lo:hi],
                        lhsT=St[:],
                        rhs=BX[:, lo:hi],
                        start=True,
                        stop=(c == 0),
                    )
                if c > 0:
                    nc.tensor.matmul(
                        out=out_p[:, 0:HLO],
                        lhsT=Etp,
                        rhs=M_s[0:D, 0:HLO],
                        start=False,
                        stop=True,
                    )
                    nc.tensor.matmul(
                        out=out_p[:, HLO:H],
                        lhsT=EtpD,
                        rhs=M_s[D:2 * D, 0:HHI],
                        start=False,
                        stop=True,
                    )

                # snapshot M for the NEXT chunk (reads M_p after this chunk's
                # update, before the next one; the Act engine runs it as soon
                # as the update's semaphore fires, independent of issue order)
                if t + 1 < NT and (t + 1) % NCH != 0:
                    M_s = mpool.tile([2 * D, HLO], bf16, name=f"M_s_{t + 1}", tag="M_s")
                    nc.scalar.copy(out=M_s[:], in_=M_p[:])

                # acc -> bf16 out tile on DVE (the Act engine stays dedicated
                # to M snapshots so its queue never backs up behind out_p);
                # grouped stores ride the otherwise-idle GpSimd queue so the
                # Sync queue's load posts never block behind compute
                if q == 0:
                    OUT4 = outpool.tile([C, G * H], bf16, name=f"OUT4_{t}", tag="OUT4")
                nc.vector.tensor_scalar_add(
                    out=OUT4[:, q * H:(q + 1) * H],
                    in0=out_p[:],
                    scalar1=0.0,
                )
                if t >= NT - G:
                    # final group: store per chunk so the drain after the
                    # last acc copy is one small transfer, not 4 chunks
                    nc.gpsimd.dma_start(
                        out=out_d[t * C:(t + 1) * C, :],
                        in_=OUT4[:, q * H:(q + 1) * H],
                    )
                elif q == G - 1:
                    t0 = t - G + 1
                    nc.gpsimd.dma_start(
                        out=out_d[t0 * C:(t + 1) * C, :].rearrange(
                            "(g p) h -> p g h", g=G
                        ),
                        in_=OUT4[:].rearrange("p (g h) -> p g h", g=G),
                    )

    # Adjacent PE matmuls often share a stationary operand (the two H-halves
    # of out1); legalization has already paired each matmul with a standalone
    # InstLdweights, so drop the redundant reloads. The key includes the PE
    # array tile position: the same weights loaded into a different array
    # quadrant is a genuine reload.
    ndropped = 0
    for blk in nc.m.functions[0].blocks:
        keep = []
        last_w = None
        for inst in blk.instructions:
            if getattr(inst, "engine", None) == mybir.EngineType.PE:
                if isinstance(inst, mybir.InstLdweights):
                    w = inst.ins[0]
                    wkey = (
                        w.memref,
                        w.offset,
                        str(w.ap),
                        str(getattr(inst, "tile_position", None)),
                        str(getattr(inst, "tile_size", None)),
                    )
                    if (
                        last_w is not None
                        and wkey == last_w
                        and not inst.has_wait()
                    ):
                        ndropped += 1
                        continue
                    last_w = wkey
                elif not isinstance(inst, mybir.InstMatmult):
                    last_w = None
            keep.append(inst)
        blk.instructions = keep
    if os.environ.get("BASS_DEBUG_FUSE"):
        print(f"[kernel] redundant ldweights dropped: {ndropped}", file=sys.stderr)

    nc.compile()
    _compiled[key] = nc
    return nc


def _np_umask():
    i = np.arange(C)
    return (i[:, None] < i[None, :]).astype(np.float32)


def _in_maps(bert_x, x, ae, w):
    import ml_dtypes

    bf16 = ml_dtypes.bfloat16
    bert_x = np.asarray(bert_x, dtype=np.float32)
    x = np.asarray(x)
    ae = np.asarray(ae, dtype=np.float32)
    w = np.asarray(w, dtype=np.float32)

    E = ae[x.reshape(-1)]                     # [B*L, D]
    A = E @ w                                 # [B*L, D]
    jp1 = (np.arange(L, dtype=np.float64) + 1.0).astype(np.float32)
    Ap = (A.reshape(B, L, D) * jp1[None, :, None]).reshape(B * L, D)
    Einv = (E.reshape(B, L, D) / jp1[None, :, None]).reshape(B * L, D)

    bx16 = np.ascontiguousarray(bert_x.reshape(B * L, H).astype(bf16))
    ap16 = np.ascontiguousarray(Ap.astype(bf16))

    # etat per core: [D, 2*ROWS]; per global chunk g: [Et'_g | At'_g]
    Ech = Einv.reshape(B, NCH, C, D).astype(bf16)
    Ach = Ap.reshape(B, NCH, C, D).astype(bf16)
    pair = np.stack([Ech, Ach], axis=2)       # [B, NCH, 2, C, D]
    pair = pair.transpose(0, 4, 1, 2, 3)      # [B, D, NCH, 2, C]

    umask = _np_umask()
    maps = []
    for k in range(NCORES):
        et = np.ascontiguousarray(
            pair[k * BPC:(k + 1) * BPC].transpose(1, 0, 2, 3, 4).reshape(D, 2 * ROWS)
        )
        maps.append(
            {
                "bx": bx16[k * BPC * L:(k + 1) * BPC * L],
                "etat": et,
                "ap": ap16[k * BPC * L:(k + 1) * BPC * L],
                "umask": umask,
            }
        )
    return maps


def _run(bert_x, x, ae, w, trace=False):
    from concourse import bass_utils

    nc = _build()
    maps = _in_maps(bert_x, x, ae, w)
    res = bass_utils.run_bass_kernel_spmd(
        nc, maps, core_ids=list(range(NCORES)), trace=trace
    )
    acc = np.concatenate(
        [
            res.results[k]["out"].astype(np.float32).reshape(BPC, L, H)
            for k in range(NCORES)
        ],
        axis=0,
    )
    out = np.asarray(bert_x, dtype=np.float32) + acc
    return out, res


def kernel(bert_x, x, ae, w):
    out, _ = _run(bert_x, x, ae, w, trace=False)
    return out
